# revision 1
# baseline (speedup 1.0000x reference)
"""AdderNet BasicBlock kernel for Trainium2, co-sharded across 8 cores.

Per core (co-shard CO=8 of 64 output channels):
  conv[co,n,p] = -sum_{ci,kh,kw} |x[n,ci,p+k-1] - w[co,ci,kh,kw]|   (pad=1)
  BN train-mode over (n,h,w) per co, then ReLU.

Formulation: |d| = 2*relu(d) - d with d = x - w, so
  conv = -2*sum(relu(x-w)) + BoxX - S_w
    BoxX = sum_{ci,tap} x_patch  (PE ones-matmuls on x directly)
    S_w  = sum_{ci,tap} w[co]    (folded into PSUM-evacuation bias)

Engines:
  DVE: tensor_scalar(sub, max 0) -> relu(x-w), fp32 2x mode
  ACT: share of relu passes (activation Relu, bias=-w) + PSUM evacuation
  PE : ones-matmul reduce over ci partitions (128 rows = 2 image groups x 64 ci),
       PSUM accumulates 9 taps; float32r for full-rate streaming
  BN : conv bounced via DRAM to [(co,n), hw]; replicated-selector matmul stats.
"""
from contextlib import ExitStack

import numpy as np

import concourse.bass as bass
import concourse.tile as tile
import concourse.mybir as mybir

F32 = mybir.dt.float32
BF16 = mybir.dt.bfloat16
F32R = mybir.dt.float32r
BN_EPS = 1e-5

N, CI, H, W = 16, 64, 32, 32
CO = 8          # output channels per core
HW = H * W      # 1024
PADH, PADW = H + 2, W + 2  # 34


def split_multiwaits(nc, max_waits=1):
    """This container's walrus rejects >1 semaphore wait per instruction.
    Hoist extras into standalone NoOps on the same (in-order) engine."""
    n_split = 0
    for f in nc.m.functions:
        for b in f.blocks:
            insts = list(b.instructions)
            changed = False
            new = []
            for inst in insts:
                si = inst.sync_info
                waits = list(si.on_wait) if si and si.on_wait else []
                if len(waits) > max_waits:
                    changed = True
                    n_split += 1
                    for w in waits[: len(waits) - max_waits]:
                        new.append(mybir.InstNoOp(
                            name=nc.get_next_instruction_name(),
                            engine=inst.engine, ins=[], outs=[],
                            sync_info=mybir.SyncInfo(on_wait=[w], on_update=[]),
                        ))
                    inst.sync_info = mybir.SyncInfo(
                        on_wait=waits[len(waits) - max_waits:],
                        on_update=list(si.on_update) if si.on_update else [],
                    )
                new.append(inst)
            if changed:
                b.instructions = new
    return n_split


def build_nc(reduce_dtype="f32r", act_tap_frac=0.3, t_cols=2048, ps_cols=2048,
             debug_out=None, cache_salt=0.0, t_bufs=8):
    """One core's SPMD program."""
    nc = bass.Bass()
    x = nc.declare_dram_parameter("x", [N, CI, H, W], F32, isOutput=False)
    w = nc.declare_dram_parameter("w", [CO, CI, 3, 3], F32, isOutput=False)
    gamma = nc.declare_dram_parameter("gamma", [CO], F32, isOutput=False)
    beta = nc.declare_dram_parameter("beta", [CO], F32, isOutput=False)
    selcor_in = nc.declare_dram_parameter("selcor", [128, 128], F32,
                                          isOutput=False)
    out = nc.declare_dram_parameter("out", [N, CO, H, W], F32, isOutput=True)

    t_dt = {"bf16": BF16, "f32r": F32R, "f32": F32}[reduce_dtype]
    n_halves = 8192 // t_cols          # spatial (within-group) split
    jph = 8 // n_halves                # images-per-group per half
    nb = ps_cols // 512                # matmul blocks per psum tile

    with tile.TileContext(nc) as tc, ExitStack() as ctx:
        singles = ctx.enter_context(tc.tile_pool(name="singles", bufs=1))
        tpool = ctx.enter_context(tc.tile_pool(name="tpool", bufs=t_bufs))
        cpool = ctx.enter_context(tc.tile_pool(name="cpool", bufs=3))
        pspool = ctx.enter_context(tc.tile_pool(name="psum", bufs=2, space="PSUM"))
        spool = ctx.enter_context(tc.tile_pool(name="stage2", bufs=1))
        dpool = ctx.enter_context(tc.tile_pool(name="dram", bufs=1, space="DRAM"))

        # ---- constants (dep-free DVE setup first) ----
        sel32 = singles.tile([128, 2], F32)         # ones-reduce weights (fp32)
        nc.vector.memset(sel32[:, :], 0.0)
        nc.vector.memset(sel32[0:64, 0:1], 1.0)
        nc.vector.memset(sel32[64:128, 1:2], 1.0)
        if t_dt == F32:
            sel = sel32
        else:
            sel = singles.tile([128, 2], t_dt)      # rounded variant for PE
            nc.vector.tensor_copy(out=sel[:, :], in_=sel32[:, :])
        eps_t = singles.tile([128, 1], F32)
        nc.vector.memset(eps_t[:, :], BN_EPS)
        if cache_salt:
            salt_t = singles.tile([8, 1], F32)
            nc.vector.memset(salt_t[:, :], cache_salt)

        w_sb = singles.tile([128, CO * 9], F32)     # w_sb[(g,ci), co*9+tap]
        w_src = w.rearrange("co ci kh kw -> ci co (kh kw)")
        nc.sync.dma_start(
            out=w_sb[0:64, :].rearrange("p (co t) -> p co t", t=9), in_=w_src)
        nc.sync.dma_start(
            out=w_sb[64:128, :].rearrange("p (co t) -> p co t", t=9), in_=w_src)

        # ---- x padded into SBUF, one tile per half: [(g,ci), j, 34, 34] ----
        auxpool = ctx.enter_context(tc.tile_pool(name="auxpool", bufs=2))
        x_pads = []
        for half in range(n_halves):
            j0 = half * jph
            xp_h = singles.tile([128, jph, PADH, PADW], F32, name=f"xpad_{half}")
            nc.vector.memset(xp_h[:, :, 0, :], 0.0)
            nc.vector.memset(xp_h[:, :, PADH - 1, :], 0.0)
            nc.vector.memset(xp_h[:, :, :, 0], 0.0)
            nc.vector.memset(xp_h[:, :, :, PADW - 1], 0.0)
            for g in range(2):
                for jj in range(jph):
                    nc.sync.dma_start(
                        out=xp_h[g * 64:(g + 1) * 64, jj, 1:H + 1, 1:W + 1],
                        in_=x[g * 8 + j0 + jj])
            x_pads.append(xp_h)

        neg_w_sb = singles.tile([128, CO * 9], F32)
        nc.vector.tensor_scalar(
            out=neg_w_sb[:, :], in0=w_sb[:, :], scalar1=-1.0, scalar2=None,
            op0=mybir.AluOpType.mult)

        # conv scratch in DRAM: [co, g, half, j_local, hw] holds -2*sum(relu)-S_w
        conv_d = dpool.tile([CO, 2, n_halves, jph, HW], F32)
        box_d = dpool.tile([2, n_halves, jph, HW], F32)

        # -S_w[co] bias for evacuation: swb [2, CO]
        wsum_sb = singles.tile([128, CO], F32)
        nc.vector.tensor_reduce(
            out=wsum_sb[:, :],
            in_=w_sb.rearrange("p (co t) -> p co t", t=9),
            axis=mybir.AxisListType.X, op=mybir.AluOpType.add)
        ps_sw = pspool.tile([2, CO], F32, tag="ps")
        nc.tensor.matmul(ps_sw[:, :], lhsT=sel32[:, :], rhs=wsum_sb[:, :],
                         start=True, stop=True)  # exact fp32, tiny
        swb = singles.tile([2, CO], F32)
        nc.scalar.mul(swb[:, :], ps_sw[:, :], -1.0)

        # stage-2 reload targets, loaded incrementally during stage 1
        cs_rl = spool.tile([128, HW], F32)      # [(co,n), hw]
        box_rl = spool.tile([128, HW], F32)     # BoxX broadcast per co

        # ---- stage 1: conv ----
        act_every = int(1.0 / act_tap_frac) if act_tap_frac > 0 else 0
        pass_idx = 0
        def emit_box(half, j0, x_aux):
            ps_box = pspool.tile([2, ps_cols], F32, tag="ps", name=f"psbox_{half}")
            for tap in range(9):
                kh, kw = divmod(tap, 3)
                for b in range(nb):
                    a, hb = divmod(b, 2)   # image-in-half, 16-row block
                    rhs = x_aux[:, a, kh + hb * 16:kh + hb * 16 + 16,
                                kw:kw + W]
                    nc.tensor.matmul(
                        ps_box[:, b * 512:(b + 1) * 512],
                        lhsT=sel[:, :], rhs=rhs,
                        start=(tap == 0), stop=(tap == 8))
            box_cs = cpool.tile([2, ps_cols], F32, tag="cs", name=f"boxcs_{half}")
            nc.scalar.copy(box_cs[:, :], ps_box[:, :])
            nc.sync.dma_start(
                out=box_d[:, half, :, :],
                in_=box_cs.rearrange("p (a hw) -> p a hw", hw=HW))

        for half in range(n_halves):
            j0 = half * jph
            x_pad = x_pads[half]
            if t_dt == F32:
                x_aux = x_pad
            else:
                x_aux = auxpool.tile([128, jph, PADH, PADW], t_dt, tag="aux",
                                     name=f"aux_{half}")
                nc.vector.tensor_copy(
                    out=x_aux.rearrange("p a h w -> p (a h w)"),
                    in_=x_pad.rearrange("p a h w -> p (a h w)"))

            for co in range(CO):
                if co == 4:
                    emit_box(half, j0, x_aux)
                ps = pspool.tile([2, ps_cols], F32, tag="ps", name=f"ps_{co}_{half}")
                for tap in range(9):
                    kh, kw = divmod(tap, 3)
                    k = co * 9 + tap
                    t = tpool.tile([128, jph, H, W], t_dt, tag="t",
                                   name=f"t_{co}_{half}_{tap}")
                    src = x_pad[:, :, kh:kh + H, kw:kw + W]
                    use_act = act_every and (pass_idx % act_every == act_every - 1)
                    pass_idx += 1
                    if use_act:
                        nc.scalar.activation(
                            out=t[:, :, :, :], in_=src,
                            func=mybir.ActivationFunctionType.Relu,
                            bias=neg_w_sb[:, k:k + 1], scale=1.0)
                    else:
                        nc.vector.tensor_scalar(
                            out=t[:, :, :, :], in0=src,
                            scalar1=w_sb[:, k:k + 1], scalar2=0.0,
                            op0=mybir.AluOpType.subtract,
                            op1=mybir.AluOpType.max)
                    tf = t.rearrange("p a h w -> p (a h w)")
                    for b in range(nb):
                        nc.tensor.matmul(
                            ps[:, b * 512:(b + 1) * 512],
                            lhsT=sel[:, :],
                            rhs=tf[:, b * 512:(b + 1) * 512],
                            start=(tap == 0), stop=(tap == 8))
                cs = cpool.tile([2, ps_cols], F32, tag="cs", name=f"cs_{co}_{half}")
                nc.scalar.activation(
                    out=cs[:, :], in_=ps[:, :],
                    func=mybir.ActivationFunctionType.Identity,
                    bias=swb[:, co:co + 1], scale=-2.0)
                nc.sync.dma_start(
                    out=conv_d[co, :, half, :, :],
                    in_=cs.rearrange("p (a hw) -> p a hw", hw=HW))
                if half == n_halves - 1:
                    nc.sync.dma_start(
                        out=cs_rl[co * 16:(co + 1) * 16, :],
                        in_=conv_d[co].rearrange("g h j w -> (g h j) w"))
                    if co == 5:
                        for c2 in range(CO):
                            nc.sync.dma_start(
                                out=box_rl[c2 * 16:(c2 + 1) * 16, :],
                                in_=box_d.rearrange("g h j w -> (g h j) w"))

        # ---- stage 2: BN stats + affine + relu ----
        selcor = singles.tile([128, 128], F32)      # replicated stats selector
        nc.sync.dma_start(out=selcor[:, :], in_=selcor_in[:, :])
        if t_dt == F32:
            selcor_r = selcor
        else:
            selcor_r = singles.tile([128, 128], t_dt)
            nc.vector.tensor_copy(out=selcor_r[:, :], in_=selcor[:, :])
        gam = singles.tile([128, 1], F32)
        bet = singles.tile([128, 1], F32)
        for co in range(CO):
            nc.sync.dma_start(out=gam[co * 16:(co + 1) * 16, :],
                              in_=gamma[co:co + 1].partition_broadcast(16))
            nc.sync.dma_start(out=bet[co * 16:(co + 1) * 16, :],
                              in_=beta[co:co + 1].partition_broadcast(16))
        conv_rl = spool.tile([128, HW], F32)    # true conv output
        nc.vector.tensor_add(conv_rl[:, :], cs_rl[:, :], box_rl[:, :])

        # stats: mean via replicated-selector matmul, then centered var
        if t_dt == F32:
            conv_r = conv_rl
        else:
            conv_r = spool.tile([128, HW], t_dt)
            nc.vector.tensor_copy(out=conv_r[:, :], in_=conv_rl[:, :])
        ps1 = pspool.tile([128, 512], F32, tag="ps")
        ps1b = pspool.tile([128, 512], F32, tag="ps")
        nc.tensor.matmul(ps1[:, :], lhsT=selcor_r[:, :], rhs=conv_r[:, 0:512],
                         start=True, stop=True)
        nc.tensor.matmul(ps1b[:, :], lhsT=selcor_r[:, :], rhs=conv_r[:, 512:HW],
                         start=True, stop=True)
        s1 = spool.tile([128, 1], F32)
        s1b = spool.tile([128, 1], F32)
        nc.vector.tensor_reduce(out=s1[:, :], in_=ps1[:, :],
                                axis=mybir.AxisListType.X, op=mybir.AluOpType.add)
        nc.vector.tensor_reduce(out=s1b[:, :], in_=ps1b[:, :],
                                axis=mybir.AxisListType.X, op=mybir.AluOpType.add)
        inv_n = 1.0 / (N * HW)
        mean = spool.tile([128, 1], F32)
        nc.vector.tensor_scalar(out=mean[:, :], in0=s1[:, :], scalar1=s1b[:, :],
                                scalar2=inv_n, op0=mybir.AluOpType.add,
                                op1=mybir.AluOpType.mult)
        # centered square -> variance without cancellation
        dctr = spool.tile([128, HW], F32)
        nc.vector.tensor_scalar(out=dctr[:, :], in0=conv_rl[:, :],
                                scalar1=mean[:, :], scalar2=None,
                                op0=mybir.AluOpType.subtract)
        sq = spool.tile([128, HW], t_dt)
        nc.scalar.activation(out=sq[:, :], in_=dctr[:, :],
                             func=mybir.ActivationFunctionType.Square)
        ps2 = pspool.tile([128, 512], F32, tag="ps")
        ps2b = pspool.tile([128, 512], F32, tag="ps")
        nc.tensor.matmul(ps2[:, :], lhsT=selcor_r[:, :], rhs=sq[:, 0:512],
                         start=True, stop=True)
        nc.tensor.matmul(ps2b[:, :], lhsT=selcor_r[:, :], rhs=sq[:, 512:HW],
                         start=True, stop=True)
        s2 = spool.tile([128, 1], F32)
        s2b = spool.tile([128, 1], F32)
        nc.vector.tensor_reduce(out=s2[:, :], in_=ps2[:, :],
                                axis=mybir.AxisListType.X, op=mybir.AluOpType.add)
        nc.vector.tensor_reduce(out=s2b[:, :], in_=ps2b[:, :],
                                axis=mybir.AxisListType.X, op=mybir.AluOpType.add)
        var = spool.tile([128, 1], F32)
        nc.vector.tensor_scalar(out=var[:, :], in0=s2[:, :], scalar1=s2b[:, :],
                                scalar2=inv_n, op0=mybir.AluOpType.add,
                                op1=mybir.AluOpType.mult)
        std = spool.tile([128, 1], F32)
        nc.scalar.activation(out=std[:, :], in_=var[:, :],
                             func=mybir.ActivationFunctionType.Sqrt,
                             bias=eps_t[:, :], scale=1.0)
        rstd = spool.tile([128, 1], F32)
        nc.vector.reciprocal(out=rstd[:, :], in_=std[:, :])
        a_t = spool.tile([128, 1], F32)
        nc.vector.tensor_mul(a_t[:, :], gam[:, :], rstd[:, :])
        ma = spool.tile([128, 1], F32)
        nc.vector.tensor_mul(ma[:, :], mean[:, :], a_t[:, :])
        b_t = spool.tile([128, 1], F32)
        nc.vector.tensor_sub(b_t[:, :], bet[:, :], ma[:, :])

        outt = spool.tile([128, HW], F32)
        if debug_out == "conv":
            nc.vector.tensor_copy(out=outt[:, :], in_=conv_rl[:, :])
        elif debug_out == "cs":
            nc.vector.tensor_copy(out=outt[:, :], in_=cs_rl[:, :])
        elif debug_out == "box":
            nc.vector.tensor_copy(out=outt[:, :], in_=box_rl[:, :])
        else:
            nc.scalar.activation(out=outt[:, :], in_=conv_rl[:, :],
                                 func=mybir.ActivationFunctionType.Relu,
                                 bias=b_t[:, :], scale=a_t[:, :])
        out_r = out.rearrange("n co h w -> co n (h w)")
        for co in range(CO):
            nc.sync.dma_start(out=out_r[co], in_=outt[co * 16:(co + 1) * 16, :])

    split_multiwaits(nc)
    return nc


def make_in_maps(x, weight, gamma, beta):
    x = np.ascontiguousarray(x, dtype=np.float32)
    weight = np.ascontiguousarray(weight, dtype=np.float32)
    gamma = np.ascontiguousarray(gamma, dtype=np.float32)
    beta = np.ascontiguousarray(beta, dtype=np.float32)
    selcor = np.zeros((128, 128), np.float32)
    for c in range(CO):
        selcor[c * 16:(c + 1) * 16, c * 16:(c + 1) * 16] = 1.0
    maps = []
    for c in range(8):
        sl = slice(c * CO, (c + 1) * CO)
        maps.append({
            "x": x,
            "w": np.ascontiguousarray(weight[sl]),
            "gamma": np.ascontiguousarray(gamma[sl]),
            "beta": np.ascontiguousarray(beta[sl]),
            "selcor": selcor,
        })
    return maps


def assemble(results):
    return np.concatenate([r["out"] for r in results], axis=1)


# ---------------------------------------------------------------------------
# Harness entry point: full inputs in, full output out.
# Sharding: output channels co split 8 ways (8 channels per NeuronCore);
# BN statistics are over the full batch, which each core owns for its
# channels, so no collectives are needed.
# ---------------------------------------------------------------------------
from concourse.bass_utils import run_bass_kernel_spmd

_NC_CACHE = None


def _get_nc():
    global _NC_CACHE
    if _NC_CACHE is None:
        _NC_CACHE = build_nc()
    return _NC_CACHE


def kernel(x, weight, gamma, beta):
    nc = _get_nc()
    in_maps = make_in_maps(np.asarray(x), np.asarray(weight),
                           np.asarray(gamma), np.asarray(beta))
    res = run_bass_kernel_spmd(nc, in_maps, core_ids=list(range(8)))
    return assemble(res.results)



# revision 4
# speedup vs baseline: 1.2722x; 1.2722x over previous
"""AdderNet BasicBlock kernel for Trainium2, co-sharded across 8 cores.

Per core (co-shard CO=8 of 64 output channels):
  conv[co,n,p] = -sum_{ci,kh,kw} |x[n,ci,p+k-1] - w[co,ci,kh,kw]|   (pad=1)
  BN train-mode over (n,h,w) per co, then ReLU.

Formulation (v2):
  Taps are split between engines:
    ACT taps (2 of 9): |d| = Abs(x + (-w)) directly  -> PE weight -1
    DVE taps (7 of 9): relu(x-w) = tensor_scalar(sub, max 0), bf16 4x mode
        |d| = 2*relu(d) - d            -> PE weight -2, Box correction +1
  conv = -(sum_ACT |d| + 2*sum_DVE relu(d)) + Box_D - SwD
  SwD is a per-channel constant -> dropped (BN is shift-invariant per channel).

PE: 4-way column tiling. Position j holds co j (tile A) / co 4+j (tile B);
ones-reduce over 128 partitions (2 image groups x 64 ci), 512-col blocks,
PSUM accumulates 9 taps. Box (7 DVE taps over x) streams into tile A after
its evacuation, at a per-half rotating position.

BN stage 2 as before: bounce conv to DRAM, reload as [(co,n), hw],
replicated-selector matmul stats, fused affine+relu on ACT.
"""
from contextlib import ExitStack

import numpy as np

import concourse.bass as bass
import concourse.tile as tile
import concourse.mybir as mybir

F32 = mybir.dt.float32
BF16 = mybir.dt.bfloat16
BN_EPS = 1e-5

N, CI, H, W = 16, 64, 32, 32
CO = 8          # output channels per core
HW = H * W      # 1024
PADH, PADW = H + 2, W + 4   # 34 x 36 (2 extra zero cols: alignment + even dims)

N_HALVES = 4
JPH = 2                      # images per group per half
TCOLS = JPH * HW             # 2048 free-dim per stream
NB = TCOLS // 512            # 512-col matmul blocks per stream

ACT_TAPS = (1, 7)            # (0,1),(2,1): kw=1 taps go to ACT (Abs direct)
# interleave ACT taps for engine overlap; DVE taps use xb0 (kw even) or xb1
TAP_ORDER = (0, 2, 3, 1, 4, 5, 7, 6, 8)


def split_multiwaits(nc, max_waits=1):
    """This container's walrus rejects >1 semaphore wait per instruction.
    Hoist extras into standalone NoOps on the same (in-order) engine."""
    n_split = 0
    for f in nc.m.functions:
        for b in f.blocks:
            insts = list(b.instructions)
            changed = False
            new = []
            for inst in insts:
                si = inst.sync_info
                waits = list(si.on_wait) if si and si.on_wait else []
                if len(waits) > max_waits:
                    changed = True
                    n_split += 1
                    for w in waits[: len(waits) - max_waits]:
                        new.append(mybir.InstNoOp(
                            name=nc.get_next_instruction_name(),
                            engine=inst.engine, ins=[], outs=[],
                            sync_info=mybir.SyncInfo(on_wait=[w], on_update=[]),
                        ))
                    inst.sync_info = mybir.SyncInfo(
                        on_wait=waits[len(waits) - max_waits:],
                        on_update=list(si.on_update) if si.on_update else [],
                    )
                new.append(inst)
            if changed:
                b.instructions = new
    return n_split


def build_nc(t_bufs=10):
    """One core's SPMD program."""
    nc = bass.Bass()
    x = nc.declare_dram_parameter("x", [N, CI, H, W], F32, isOutput=False)
    w = nc.declare_dram_parameter("w", [CO, CI, 3, 3], F32, isOutput=False)
    gamma = nc.declare_dram_parameter("gamma", [CO], F32, isOutput=False)
    beta = nc.declare_dram_parameter("beta", [CO], F32, isOutput=False)
    selcor_in = nc.declare_dram_parameter("selcor", [128, 128], F32,
                                          isOutput=False)
    out = nc.declare_dram_parameter("out", [N, CO, H, W], F32, isOutput=True)

    with tile.TileContext(nc) as tc, ExitStack() as ctx:
        singles = ctx.enter_context(tc.tile_pool(name="singles", bufs=1))
        xspool = ctx.enter_context(tc.tile_pool(name="xstage", bufs=2))
        xbpool = ctx.enter_context(tc.tile_pool(name="xb", bufs=2))
        tpool = ctx.enter_context(tc.tile_pool(name="tpool", bufs=t_bufs))
        scpool = ctx.enter_context(tc.tile_pool(name="scr", bufs=2))
        sbpool = ctx.enter_context(tc.tile_pool(name="scrbox", bufs=2))
        pspool = ctx.enter_context(tc.tile_pool(name="psum", bufs=2,
                                                space="PSUM"))
        spool = ctx.enter_context(tc.tile_pool(name="stage2", bufs=1))
        dpool = ctx.enter_context(tc.tile_pool(name="dram", bufs=1,
                                               space="DRAM"))

        # ---- constants ----
        def ones_pair(val, nm):
            t = singles.tile([128, 2], BF16, name=nm)
            nc.vector.memset(t[:, :], 0.0)
            nc.vector.memset(t[0:64, 0:1], val)
            nc.vector.memset(t[64:128, 1:2], val)
            return t
        sel_m2 = ones_pair(-2.0, "sel_m2")   # DVE relu streams
        sel_m1 = ones_pair(-1.0, "sel_m1")   # ACT |d| streams
        sel_p1 = ones_pair(1.0, "sel_p1")    # box streams

        eps_t = singles.tile([128, 1], F32)
        nc.vector.memset(eps_t[:, :], BN_EPS)

        w_sb = singles.tile([128, CO * 9], F32)     # w_sb[(g,ci), co*9+tap]
        w_src = w.rearrange("co ci kh kw -> ci co (kh kw)")
        nc.sync.dma_start(
            out=w_sb[0:64, :].rearrange("p (co t) -> p co t", t=9), in_=w_src)
        nc.sync.dma_start(
            out=w_sb[64:128, :].rearrange("p (co t) -> p co t", t=9), in_=w_src)
        neg_w_sb = singles.tile([128, CO * 9], F32)
        nc.vector.tensor_scalar(
            out=neg_w_sb[:, :], in0=w_sb[:, :], scalar1=-1.0, scalar2=None,
            op0=mybir.AluOpType.mult)

        # conv scratch in DRAM: [co, g, half, j, hw] holds S
        conv_d = dpool.tile([CO, 2, N_HALVES, JPH, HW], F32)
        box_d = dpool.tile([2, N_HALVES, JPH, HW], F32)

        # stage-2 reload targets, loaded incrementally during stage 1
        cs_rl = spool.tile([128, HW], F32)      # S, [(co,n), hw]
        box_rl = spool.tile([128, HW], F32)     # Box broadcast per co

        # ---- stage 1 ----
        for half in range(N_HALVES):
            j0 = half * JPH
            # padded fp32 staging
            x_st = xspool.tile([128, JPH, PADH, PADW], F32, tag="xst",
                               name=f"xst{half}")
            nc.vector.memset(x_st[:, :, 0, :], 0.0)
            nc.vector.memset(x_st[:, :, PADH - 1, :], 0.0)
            nc.vector.memset(x_st[:, :, 1:PADH - 1, 0:1], 0.0)
            nc.vector.memset(x_st[:, :, 1:PADH - 1, H + 1:PADW], 0.0)
            for g in range(2):
                for jj in range(JPH):
                    nc.sync.dma_start(
                        out=x_st[g * 64:(g + 1) * 64, jj, 1:H + 1, 1:W + 1],
                        in_=x[g * 8 + j0 + jj])
            # bf16 copies: xb0 aligned for even kw, xb1 shifted left 1 col
            xb0 = xbpool.tile([128, JPH, PADH, PADW], BF16, tag="xb0",
                              name=f"xb0_{half}")
            nc.vector.tensor_copy(
                out=xb0.rearrange("p a h w -> p (a h w)"),
                in_=x_st.rearrange("p a h w -> p (a h w)"))
            xb1 = xbpool.tile([128, JPH, PADH, PADW], BF16, tag="xb1",
                              name=f"xb1_{half}")
            nc.vector.tensor_copy(
                out=xb1[:, :, :, 0:PADW - 2],
                in_=x_st[:, :, :, 1:PADW - 1])

            def tap_src(tap):
                kh, kw = divmod(tap, 3)
                if kw == 1 and tap not in ACT_TAPS:
                    return xb1, kh, 0        # shifted copy, aligned
                return xb0, kh, kw

            psA = pspool.tile([128, TCOLS], F32, tag="ps", name=f"psA{half}")
            psB = pspool.tile([128, TCOLS], F32, tag="ps", name=f"psB{half}")

            for slot, (ps, co0) in enumerate(((psA, 0), (psB, 4))):
                for ti, tap in enumerate(TAP_ORDER):
                    kh, kw = divmod(tap, 3)
                    src_t, skh, skw = tap_src(tap)
                    on_act = tap in ACT_TAPS
                    sel = sel_m1 if on_act else sel_m2
                    ts = []
                    for cl in range(4):
                        co = co0 + cl
                        k = co * 9 + tap
                        t = tpool.tile([128, JPH, H, W], BF16, tag="t",
                                       name=f"t{half}_{slot}_{ti}_{cl}")
                        src = src_t[:, :, skh:skh + H, skw:skw + W]
                        if on_act:
                            nc.scalar.activation(
                                out=t[:, :, :, :], in_=src,
                                func=mybir.ActivationFunctionType.Abs,
                                bias=neg_w_sb[:, k:k + 1], scale=1.0)
                        else:
                            nc.vector.tensor_scalar(
                                out=t[:, :, :, :], in0=src,
                                scalar1=w_sb[:, k:k + 1], scalar2=0.0,
                                op0=mybir.AluOpType.subtract,
                                op1=mybir.AluOpType.max)
                        ts.append(t.rearrange("p a h w -> p (a h w)"))
                    for b in range(NB):
                        for cl in range(4):
                            nc.tensor.matmul(
                                ps[32 * cl:32 * cl + 2,
                                   b * 512:(b + 1) * 512],
                                lhsT=sel[:, :],
                                rhs=ts[cl][:, b * 512:(b + 1) * 512],
                                start=(ti == 0), stop=(ti == 8),
                                tile_position=(0, 32 * cl))

                # evacuate + DMA out this slot's 4 channels
                scr = scpool.tile([128, TCOLS], F32, tag="scr",
                                  name=f"scr{half}_{slot}")
                nc.scalar.copy(scr[:, :], ps[:, :])
                for cl in range(4):
                    co = co0 + cl
                    nc.sync.dma_start(
                        out=conv_d[co, :, half, :, :],
                        in_=scr[32 * cl:32 * cl + 2, :].rearrange(
                            "p (a hw) -> p a hw", hw=HW))
                    if half == N_HALVES - 1:
                        nc.sync.dma_start(
                            out=cs_rl[co * 16:(co + 1) * 16, :],
                            in_=conv_d[co].rearrange("g h j w -> (g h j) w"))

                if slot == 0:
                    # box streams reuse tile A rows at a rotating position
                    bp = half % 4
                    box_taps = [t for t in range(9) if t not in ACT_TAPS]
                    for bi, tap in enumerate(box_taps):
                        src_t, skh, skw = tap_src(tap)
                        for b in range(NB):
                            a, hb = divmod(b, 2)
                            rhs = src_t[:, a, skh + hb * 16:skh + hb * 16 + 16,
                                        skw:skw + W]
                            nc.tensor.matmul(
                                psA[32 * bp:32 * bp + 2,
                                    b * 512:(b + 1) * 512],
                                lhsT=sel_p1[:, :], rhs=rhs,
                                start=(bi == 0), stop=(bi == len(box_taps) - 1),
                                tile_position=(0, 32 * bp))
                    scb = sbpool.tile([2, TCOLS], F32, tag="scb",
                                      name=f"scb{half}")
                    nc.vector.tensor_copy(
                        out=scb[:, :], in_=psA[32 * bp:32 * bp + 2, :])
                    nc.sync.dma_start(
                        out=box_d[:, half, :, :],
                        in_=scb.rearrange("p (a hw) -> p a hw", hw=HW))
                    if half == N_HALVES - 1:
                        for c2 in range(CO):
                            nc.sync.dma_start(
                                out=box_rl[c2 * 16:(c2 + 1) * 16, :],
                                in_=box_d.rearrange("g h j w -> (g h j) w"))

        # ---- stage 2: BN stats + affine + relu ----
        selcor = singles.tile([128, 128], F32)      # replicated stats selector
        nc.sync.dma_start(out=selcor[:, :], in_=selcor_in[:, :])
        selcor_r = singles.tile([128, 128], BF16)
        nc.vector.tensor_copy(out=selcor_r[:, :], in_=selcor[:, :])
        gam = singles.tile([128, 1], F32)
        bet = singles.tile([128, 1], F32)
        for co in range(CO):
            nc.sync.dma_start(out=gam[co * 16:(co + 1) * 16, :],
                              in_=gamma[co:co + 1].partition_broadcast(16))
            nc.sync.dma_start(out=bet[co * 16:(co + 1) * 16, :],
                              in_=beta[co:co + 1].partition_broadcast(16))
        conv_rl = spool.tile([128, HW], F32)    # true conv output (+SwD shift)
        nc.vector.tensor_add(conv_rl[:, :], cs_rl[:, :], box_rl[:, :])

        # stats: mean via replicated-selector matmul, then centered var
        conv_r = spool.tile([128, HW], BF16)
        nc.vector.tensor_copy(out=conv_r[:, :], in_=conv_rl[:, :])
        ps1 = pspool.tile([128, 512], F32, tag="ps")
        ps1b = pspool.tile([128, 512], F32, tag="ps")
        nc.tensor.matmul(ps1[:, :], lhsT=selcor_r[:, :], rhs=conv_r[:, 0:512],
                         start=True, stop=True)
        nc.tensor.matmul(ps1b[:, :], lhsT=selcor_r[:, :], rhs=conv_r[:, 512:HW],
                         start=True, stop=True)
        s1 = spool.tile([128, 1], F32)
        s1b = spool.tile([128, 1], F32)
        nc.vector.tensor_reduce(out=s1[:, :], in_=ps1[:, :],
                                axis=mybir.AxisListType.X, op=mybir.AluOpType.add)
        nc.vector.tensor_reduce(out=s1b[:, :], in_=ps1b[:, :],
                                axis=mybir.AxisListType.X, op=mybir.AluOpType.add)
        inv_n = 1.0 / (N * HW)
        mean = spool.tile([128, 1], F32)
        nc.vector.tensor_scalar(out=mean[:, :], in0=s1[:, :], scalar1=s1b[:, :],
                                scalar2=inv_n, op0=mybir.AluOpType.add,
                                op1=mybir.AluOpType.mult)
        # centered square -> variance without cancellation
        dctr = spool.tile([128, HW], F32)
        nc.vector.tensor_scalar(out=dctr[:, :], in0=conv_rl[:, :],
                                scalar1=mean[:, :], scalar2=None,
                                op0=mybir.AluOpType.subtract)
        sq = spool.tile([128, HW], BF16)
        nc.scalar.activation(out=sq[:, :], in_=dctr[:, :],
                             func=mybir.ActivationFunctionType.Square)
        ps2 = pspool.tile([128, 512], F32, tag="ps")
        ps2b = pspool.tile([128, 512], F32, tag="ps")
        nc.tensor.matmul(ps2[:, :], lhsT=selcor_r[:, :], rhs=sq[:, 0:512],
                         start=True, stop=True)
        nc.tensor.matmul(ps2b[:, :], lhsT=selcor_r[:, :], rhs=sq[:, 512:HW],
                         start=True, stop=True)
        s2 = spool.tile([128, 1], F32)
        s2b = spool.tile([128, 1], F32)
        nc.vector.tensor_reduce(out=s2[:, :], in_=ps2[:, :],
                                axis=mybir.AxisListType.X, op=mybir.AluOpType.add)
        nc.vector.tensor_reduce(out=s2b[:, :], in_=ps2b[:, :],
                                axis=mybir.AxisListType.X, op=mybir.AluOpType.add)
        var = spool.tile([128, 1], F32)
        nc.vector.tensor_scalar(out=var[:, :], in0=s2[:, :], scalar1=s2b[:, :],
                                scalar2=inv_n, op0=mybir.AluOpType.add,
                                op1=mybir.AluOpType.mult)
        std = spool.tile([128, 1], F32)
        nc.scalar.activation(out=std[:, :], in_=var[:, :],
                             func=mybir.ActivationFunctionType.Sqrt,
                             bias=eps_t[:, :], scale=1.0)
        rstd = spool.tile([128, 1], F32)
        nc.vector.reciprocal(out=rstd[:, :], in_=std[:, :])
        a_t = spool.tile([128, 1], F32)
        nc.vector.tensor_mul(a_t[:, :], gam[:, :], rstd[:, :])
        ma = spool.tile([128, 1], F32)
        nc.vector.tensor_mul(ma[:, :], mean[:, :], a_t[:, :])
        b_t = spool.tile([128, 1], F32)
        nc.vector.tensor_sub(b_t[:, :], bet[:, :], ma[:, :])

        outt = spool.tile([128, HW], F32)
        nc.scalar.activation(out=outt[:, :], in_=conv_rl[:, :],
                             func=mybir.ActivationFunctionType.Relu,
                             bias=b_t[:, :], scale=a_t[:, :])
        out_r = out.rearrange("n co h w -> co n (h w)")
        for co in range(CO):
            nc.sync.dma_start(out=out_r[co], in_=outt[co * 16:(co + 1) * 16, :])

    split_multiwaits(nc)
    return nc


def make_in_maps(x, weight, gamma, beta):
    x = np.ascontiguousarray(x, dtype=np.float32)
    weight = np.ascontiguousarray(weight, dtype=np.float32)
    gamma = np.ascontiguousarray(gamma, dtype=np.float32)
    beta = np.ascontiguousarray(beta, dtype=np.float32)
    selcor = np.zeros((128, 128), np.float32)
    for c in range(CO):
        selcor[c * 16:(c + 1) * 16, c * 16:(c + 1) * 16] = 1.0
    maps = []
    for c in range(8):
        sl = slice(c * CO, (c + 1) * CO)
        maps.append({
            "x": x,
            "w": np.ascontiguousarray(weight[sl]),
            "gamma": np.ascontiguousarray(gamma[sl]),
            "beta": np.ascontiguousarray(beta[sl]),
            "selcor": selcor,
        })
    return maps


def assemble(results):
    return np.concatenate([r["out"] for r in results], axis=1)


# ---------------------------------------------------------------------------
# Harness entry point: full inputs in, full output out.
# Sharding: output channels co split 8 ways (8 channels per NeuronCore);
# BN statistics are over the full batch, which each core owns for its
# channels, so no collectives are needed.
# ---------------------------------------------------------------------------
from concourse.bass_utils import run_bass_kernel_spmd

_NC_CACHE = None


def _get_nc():
    global _NC_CACHE
    if _NC_CACHE is None:
        _NC_CACHE = build_nc()
    return _NC_CACHE


def kernel(x, weight, gamma, beta):
    nc = _get_nc()
    in_maps = make_in_maps(np.asarray(x), np.asarray(weight),
                           np.asarray(gamma), np.asarray(beta))
    res = run_bass_kernel_spmd(nc, in_maps, core_ids=list(range(8)))
    return assemble(res.results)
